# revision 13
# baseline (speedup 1.0000x reference)
"""Trainium2 Bass kernel for a 2-layer GAT (PyG GATConv semantics).

Strategy (8 NeuronCores, SPMD, 2 launches = 1 per GAT layer):
  - Destinations sharded across cores (6272 per core incl. padding dsts),
    destinations degree-sorted so per-tile slot grids pad tightly.
  - NO device-side gather. The host slot-expands the layer input:
    xTsl[:, b*128 + j] = x^T column of the source of edge slot (b, j),
    where block b = (tile t, slot level l) and partition j = destination
    lane. Slot level 0 is the self-loop (PyG add_self_loops); levels
    1..deg are the in-edges; the rest are zero-padded (masked).
  - Each 128-column block becomes one PE matmul lhsT against
    wext = [W | W@a_src^T | W@a_dst^T], producing full per-edge rows
    [h | alpha_src | alpha_dst-of-src] directly in PSUM -- the same
    trick the previous version used for self-loop rows only, now for
    every edge. PSUM blocks are copied (batched per bank) to SBUF.
  - Attention: e = alpha_src(slot) + alpha_dst(dst) (dst alphas from a
    small per-tile matmul over own columns), w = exp(lrelu(e)) * mask;
    softmax is deferred: DVE reduces w and w*h over the slot axis, then
    one reciprocal multiply normalizes; + bias (+ relu for layer 1).
  - Between layers the host assembles h1, casts to fp16 and re-expands
    the SAME slot grid (graph is static), so layer 2 is identical with
    H=1, C=64.
"""

import sys

for _p in ("/opt/trn_rl_repo", "/root/.axon_site/_ro/trn_rl_repo"):
    if _p not in sys.path:
        sys.path.insert(0, _p)

import os
from contextlib import ExitStack

import numpy as np

import concourse.tile as tile
from concourse import bacc, mybir
from concourse.bass_utils import run_bass_kernel_spmd

# set GAT_TRACE=1 to profile each launch; exec times land in LAST_EXEC_NS
LAST_EXEC_NS = []
LAST_RES = []

CFG = {
    "group": 16,       # max tiles per group
    "blk_budget": 96,  # max T*L blocks per group (SBUF bound)
    "xsl_bufs": 2,
    "gpool_bufs": 2,
    "epool_bufs": 3,
    "psum_bufs": 3,  # bpsum tiles are 2 banks each
    "opsum_bufs": 2,
}

f32 = mybir.dt.float32
f16 = mybir.dt.float16

P = 128
NEG_SLOPE = 0.2
N_NODES = 50000
N_CORES = 8


# ---------------------------------------------------------------- host routing


class SlotPlan:
    """Destination-sharded slot grid; slot 0 = self-loop, then in-edges."""

    def __init__(self, src, dst, n_nodes, n_cores, group, blk_budget):
        self.n_nodes = n_nodes
        self.n_cores = n_cores
        self.dpc = int(np.ceil(n_nodes / n_cores / P)) * P
        self.nt = self.dpc // P
        nt = self.nt

        src = np.asarray(src, dtype=np.int64)
        dst = np.asarray(dst, dtype=np.int64)

        self.cores = []
        Ls = np.zeros(nt, np.int64)
        for c in range(n_cores):
            lo, hi = c * self.dpc, (c + 1) * self.dpc
            m = (dst >= lo) & (dst < hi)
            d_loc = (dst[m] - lo).astype(np.int64)
            s = src[m].astype(np.int64)
            order = np.argsort(d_loc, kind="stable")
            d_loc, s = d_loc[order], s[order]
            deg = np.bincount(d_loc, minlength=self.dpc)
            offs = np.zeros(self.dpc + 1, np.int64)
            np.cumsum(deg, out=offs[1:])
            perm = np.argsort(-deg, kind="stable").astype(np.int64)
            self.cores.append(dict(deg=deg, offs=offs, srcs=s, perm=perm))
            pt = deg[perm].reshape(nt, P)
            np.maximum(Ls, pt.max(axis=1) + 1, out=Ls)  # +1 self slot

        # SPMD-uniform groups: (g0, T) tiles sharing slot depth Lg
        self.groups = []
        t0 = 0
        while t0 < nt:
            T = 1
            while (
                T < group and t0 + T < nt
                and (T + 1) * int(Ls[t0:t0 + T + 1].max()) <= blk_budget
            ):
                T += 1
            self.groups.append((t0, T))
            t0 += T
        self.Lg = {g0: int(Ls[g0:g0 + T].max()) for g0, T in self.groups}
        self.n_blocks = sum(T * self.Lg[g0] for g0, T in self.groups)
        self.S = self.n_blocks * P  # total slot columns per core
        # block start offset per group
        self.gblk = {}
        b = 0
        for g0, T in self.groups:
            self.gblk[g0] = b
            b += T * self.Lg[g0]

        # per-core slot->node map (-1 = pad) and validity mask
        self.col_node = []
        self.masks = []
        for c in range(n_cores):
            st = self.cores[c]
            deg, offs, srcs, perm = (st["deg"], st["offs"], st["srcs"],
                                     st["perm"])
            cn = np.full((self.n_blocks, P), -1, np.int64)
            for g0, T in self.groups:
                L = self.Lg[g0]
                b0 = self.gblk[g0]
                for t in range(T):
                    dsts = perm[(g0 + t) * P:(g0 + t + 1) * P]
                    node = c * self.dpc + dsts
                    ok = node < n_nodes
                    # slot 0: self column (pad dst -> -1 column = zeros)
                    cn[b0 + t * L, ok] = node[ok]
                    for j in range(P):
                        d = dsts[j]
                        dd = deg[d]
                        if dd:
                            o = offs[d]
                            cn[b0 + t * L + 1:b0 + t * L + 1 + dd, j] = \
                                srcs[o:o + dd]
            self.col_node.append(cn)
            mask = (cn >= 0).astype(np.float16)
            # self slots of pad dsts: keep 1 so den=exp(0)=1 (row dropped)
            for g0, T in self.groups:
                L = self.Lg[g0]
                b0 = self.gblk[g0]
                for t in range(T):
                    mask[b0 + t * L, :] = 1.0
            self.masks.append(np.ascontiguousarray(mask.T))  # [P, n_blocks]

    def expand(self, core, x_t):
        """[128, S] f16: x^T columns in slot order; pad -> 0."""
        cn = self.col_node[core].reshape(-1)
        out = np.zeros((x_t.shape[0], cn.size), np.float16)
        ok = cn >= 0
        out[:, ok] = x_t[:, cn[ok]]
        return out

    def xtown(self, core, x_t):
        """[128, dpc] f16: own dst columns (A-order) for alpha_dst."""
        st = self.cores[core]
        node = core * self.dpc + st["perm"]
        valid = node < self.n_nodes
        out = np.zeros((x_t.shape[0], self.dpc), np.float16)
        out[:, valid] = x_t[:, node[valid]]
        return out

    def unpermute(self, core_outs, fout):
        full = np.zeros((self.n_nodes, fout), np.float32)
        for c, arr in enumerate(core_outs):
            node = c * self.dpc + self.cores[c]["perm"]
            m = node < self.n_nodes
            full[node[m]] = arr[m]
        return full


# ------------------------------------------------------------- device program


def build_layer_program(plan: SlotPlan, n_heads, ch, relu, n_cores):
    """One GAT layer over host-expanded slot columns. Returns compiled Bacc."""
    outf = n_heads * ch
    rowv = outf + n_heads  # built rows: [h | alpha_src]
    wcols = outf + 2 * n_heads  # wext input: [W | a_src | a_dst]
    nt = plan.nt
    H, C = n_heads, ch
    bank_blocks = 512 // rowv   # blocks per 512-col psum bank
    seg = bank_blocks * rowv    # cols used per bank
    per_tile = 2 * bank_blocks  # blocks per 2-bank psum tile

    nc = bacc.Bacc(
        "TRN2",
        target_bir_lowering=False,
        debug=False,
        num_devices=n_cores,
    )
    xsl = nc.dram_tensor("xsl", [P, plan.S], f16, kind="ExternalInput").ap()
    xtown = nc.dram_tensor("xtown", [P, plan.dpc], f16,
                           kind="ExternalInput").ap()
    wext = nc.dram_tensor("wext", [P, wcols], f16, kind="ExternalInput").ap()
    maskin = nc.dram_tensor("mask", [P, plan.n_blocks], f16,
                            kind="ExternalInput").ap()
    bias = nc.dram_tensor("bias", [P, outf], f32, kind="ExternalInput").ap()
    ident_in = nc.dram_tensor("ident", [P, P], f16, kind="ExternalInput").ap()
    out = nc.dram_tensor("out", [plan.dpc, outf], f32,
                         kind="ExternalOutput").ap()
    tiles_per_bank = 512 // outf  # num-psum tiles per bank

    with tile.TileContext(nc) as tc, ExitStack() as ctx:
        const = ctx.enter_context(tc.tile_pool(name="const", bufs=1))
        xpool = ctx.enter_context(
            tc.tile_pool(name="xpool", bufs=CFG["xsl_bufs"]))
        gpool = ctx.enter_context(
            tc.tile_pool(name="gpool", bufs=CFG["gpool_bufs"]))
        epool = ctx.enter_context(
            tc.tile_pool(name="epool", bufs=CFG["epool_bufs"]))
        bpsum = ctx.enter_context(
            tc.tile_pool(name="bpsum", bufs=CFG["psum_bufs"], space="PSUM"))
        npsum = ctx.enter_context(
            tc.tile_pool(name="npsum", bufs=CFG["opsum_bufs"], space="PSUM"))

        ident = const.tile([P, P], f16)
        nc.sync.dma_start(out=ident[:], in_=ident_in[:])
        wext_sb = const.tile([P, wcols], f16)
        nc.sync.dma_start(out=wext_sb[:], in_=wext[:])
        bias_sb = const.tile([P, outf], f32)
        nc.sync.dma_start(out=bias_sb[:], in_=bias[:])
        mask_sb = const.tile([P, plan.n_blocks], f16)
        nc.sync.dma_start(out=mask_sb[:], in_=maskin[:])
        xtown_sb = const.tile([P, plan.dpc], f16)
        nc.sync.dma_start(out=xtown_sb[:], in_=xtown[:])

        # ---- OWND: alpha_dst of own dsts [P, nt*H] (one psum bank)
        OWND = const.tile([P, nt * H], f16)
        ps_d = npsum.tile([P, 512], f32, space="PSUM", tag="nps")
        for k in range(nt):
            nc.tensor.matmul(
                out=ps_d[:, k * H:(k + 1) * H],
                lhsT=xtown_sb[:, k * P:(k + 1) * P],
                rhs=wext_sb[:, outf + H:outf + 2 * H],
                start=True, stop=True,
            )
        nc.vector.tensor_copy(out=OWND[:], in_=ps_d[:, :nt * H])
        OWND3 = OWND[:].rearrange("p (t h) -> p t h", t=nt, h=H)

        # ---- per-group pipeline
        for g0, T in plan.groups:
            L = plan.Lg[g0]
            nblk = T * L
            b0 = plan.gblk[g0]

            xch = xpool.tile([P, nblk * P], f16, tag="xch")
            nc.sync.dma_start(
                out=xch[:], in_=xsl[:, b0 * P:(b0 + nblk) * P])

            G = gpool.tile([P, nblk * rowv], f16, tag="G")
            # build rows: one matmul per block, batched per psum bank
            b = 0
            copy_tog = 0
            while b < nblk:
                bn = min(per_tile, nblk - b)
                ps = bpsum.tile([P, 1024], f32, space="PSUM", tag="bps")
                for k in range(bn):
                    bank, off = divmod(k, bank_blocks)
                    o0 = bank * 512 + off * rowv
                    nc.tensor.matmul(
                        out=ps[:, o0:o0 + rowv],
                        lhsT=xch[:, (b + k) * P:(b + k + 1) * P],
                        rhs=wext_sb[:, :rowv],
                        start=True, stop=True,
                    )
                eng = nc.vector.tensor_copy if copy_tog % 4 == 0 else None
                if bn == per_tile:
                    dst = G[:, b * rowv:(b + bn) * rowv].rearrange(
                        "p (s v) -> p s v", s=2, v=seg)
                    srcv = ps[:].rearrange(
                        "p (s q) -> p s q", s=2, q=512)[:, :, :seg]
                    if eng:
                        eng(out=dst, in_=srcv)
                    else:
                        nc.scalar.copy(dst, srcv)
                else:
                    k0 = 0
                    while k0 < bn:
                        kn = min(bank_blocks, bn - k0)
                        bank = k0 // bank_blocks
                        dst = G[:, (b + k0) * rowv:(b + k0 + kn) * rowv]
                        srcv = ps[:, bank * 512:bank * 512 + kn * rowv]
                        if eng:
                            eng(out=dst, in_=srcv)
                        else:
                            nc.scalar.copy(dst, srcv)
                        k0 += kn
                copy_tog += 1
                b += bn

            G4 = G[:].rearrange("p (t l v) -> p t l v", t=T, l=L, v=rowv)

            # ---- attention weights w = exp(lrelu(a_src + a_dst)) * mask
            E = epool.tile([P, T * L * H], f16, tag="E")
            E4 = E[:].rearrange("p (t l h) -> p t l h", t=T, l=L, h=H)
            nc.vector.tensor_tensor(
                out=E4,
                in0=G4[:, :, :, outf:outf + H],
                in1=OWND3[:, g0:g0 + T, :].unsqueeze(2)
                .to_broadcast([P, T, L, H]),
                op=mybir.AluOpType.add,
            )
            W = epool.tile([P, T * L * H], f16, tag="W")
            nc.vector.scalar_tensor_tensor(
                out=W[:], in0=E[:], scalar=NEG_SLOPE, in1=E[:],
                op0=mybir.AluOpType.mult, op1=mybir.AluOpType.max,
            )
            nc.scalar.activation(W[:], W[:], mybir.ActivationFunctionType.Exp)
            W4 = W[:].rearrange("p (t l h) -> p t l h", t=T, l=L, h=H)
            nc.vector.tensor_tensor(
                out=W4, in0=W4,
                in1=mask_sb[:, b0:b0 + nblk]
                .rearrange("p (t l) -> p t l", t=T, l=L)
                .unsqueeze(3).to_broadcast([P, T, L, H]),
                op=mybir.AluOpType.mult,
            )

            # ---- denominators + reciprocal
            den = epool.tile([P, T * H], f32, tag="den")
            den3 = den[:].rearrange("p (t h) -> p t h", t=T, h=H)
            nc.vector.tensor_reduce(
                out=den3, in_=W4.transpose([0, 1, 3, 2]),
                axis=mybir.AxisListType.X, op=mybir.AluOpType.add,
            )
            rec = epool.tile([P, T * H], f32, tag="rec")
            nc.vector.reciprocal(rec[:], den[:])
            rec3 = rec[:].rearrange("p (t h) -> p t h", t=T, h=H)

            # ---- weighted sum of h over slots: scale on DVE, reduce on PE
            gh4 = G4[:, :, :, :outf].rearrange(
                "p t l (c h) -> p t l c h", c=C, h=H)
            nc.vector.tensor_tensor(
                out=gh4, in0=gh4,
                in1=W4.unsqueeze(3).to_broadcast([P, T, L, C, H]),
                op=mybir.AluOpType.mult,
            )
            osb = epool.tile([P, T * outf], f32, tag="osb")
            osb3 = osb[:].rearrange("p (t f) -> p t f", t=T, f=outf)
            G3 = G[:].rearrange("p (b v) -> p b v", b=T * L, v=rowv)
            t0 = 0
            while t0 < T:
                tn = min(tiles_per_bank, T - t0)
                ps = npsum.tile([P, 512], f32, space="PSUM", tag="nps")
                for tt in range(tn):
                    for j in range(L):
                        nc.tensor.matmul(
                            out=ps[:, tt * outf:(tt + 1) * outf],
                            lhsT=ident[:],
                            rhs=G3[:, (t0 + tt) * L + j, :outf],
                            start=(j == 0), stop=(j == L - 1),
                        )
                # normalize from PSUM: osb = num * (1/den)
                nc.vector.tensor_tensor(
                    out=osb3[:, t0:t0 + tn, :].rearrange(
                        "p t (c h) -> p t c h", c=C, h=H),
                    in0=ps[:, :tn * outf].rearrange(
                        "p (t c h) -> p t c h", t=tn, c=C, h=H),
                    in1=rec3[:, t0:t0 + tn, :].unsqueeze(2)
                    .to_broadcast([P, tn, C, H]),
                    op=mybir.AluOpType.mult,
                )
                t0 += tn

            # ---- bias (+ relu), write out
            nc.vector.tensor_tensor(
                out=osb3, in0=osb3,
                in1=bias_sb[:].unsqueeze(1).to_broadcast([P, T, outf]),
                op=mybir.AluOpType.add,
            )
            if relu:
                nc.scalar.activation(osb[:], osb[:],
                                     mybir.ActivationFunctionType.Relu)
            nc.sync.dma_start(
                out=out[g0 * P:(g0 + T) * P, :].rearrange(
                    "(t p) f -> p t f", t=T),
                in_=osb3,
            )

    nc.compile()
    return nc


# ------------------------------------------------------------------ execution


def _prep_wext(W, att_src, att_dst):
    """[fin, outf + 2H] fp16: [W (c-major cols) | W@a_src^T | W@a_dst^T]."""
    H, C = att_src.shape
    fin = W.shape[0]
    Wr = W.reshape(fin, H, C)
    a_s = np.einsum("fhc,hc->fh", Wr, att_src)
    a_d = np.einsum("fhc,hc->fh", Wr, att_dst)
    Wi = Wr.transpose(0, 2, 1).reshape(fin, H * C)  # (c, h) column order
    return np.concatenate([Wi, a_s, a_d], axis=1).astype(np.float16)


def _interleave_cols(v, H, C):
    return np.asarray(v, np.float32).reshape(H, C).T.reshape(H * C)


def _deinterleave(arr, H, C):
    n = arr.shape[0]
    return arr.reshape(n, C, H).transpose(0, 2, 1).reshape(n, H * C)


def run_layer(plan, nc, x_t, W, att_src, att_dst, b, n_cores):
    H, C = att_src.shape
    outf = H * C
    wext = _prep_wext(np.asarray(W, np.float32),
                      np.asarray(att_src, np.float32),
                      np.asarray(att_dst, np.float32))
    bias = np.broadcast_to(_interleave_cols(b, H, C), (P, outf)).copy()
    ident = np.eye(P, dtype=np.float16)
    in_maps = [
        {"xsl": plan.expand(c, x_t), "xtown": plan.xtown(c, x_t),
         "wext": wext, "bias": bias, "mask": plan.masks[c], "ident": ident}
        for c in range(n_cores)
    ]
    trace = os.environ.get("GAT_TRACE", "") == "1"
    res = run_bass_kernel_spmd(nc, in_maps, list(range(n_cores)), trace=trace)
    if trace:
        LAST_EXEC_NS.append(res.exec_time_ns)
        LAST_RES.append(res)
    outs = [res.results[c]["out"] for c in range(n_cores)]
    return _deinterleave(plan.unpermute(outs, outf), H, C)


def gat_forward(x, edge_index, params, n_cores=N_CORES):
    x = np.asarray(x, np.float32)
    n = x.shape[0]
    ei = np.asarray(edge_index)

    plan = SlotPlan(ei[0], ei[1], n, n_cores, CFG["group"],
                    CFG["blk_budget"])
    W1, as1, ad1, b1, W2, as2, ad2, b2 = params

    x_t = x.T.astype(np.float16)
    nc1 = build_layer_program(plan, as1.shape[0], as1.shape[1],
                              relu=True, n_cores=n_cores)
    h = run_layer(plan, nc1, x_t, W1, as1, ad1, b1, n_cores)

    h_t = h.T.astype(np.float16)
    nc2 = build_layer_program(plan, as2.shape[0], as2.shape[1],
                              relu=False, n_cores=n_cores)
    out = run_layer(plan, nc2, h_t, W2, as2, ad2, b2, n_cores)
    return out


def kernel(x, edge_index, W1, att_src1, att_dst1, b1, W2, att_src2,
           att_dst2, b2):
    params = tuple(
        np.asarray(a, np.float32)
        for a in (W1, att_src1, att_dst1, b1, W2, att_src2, att_dst2, b2)
    )
    return gat_forward(x, edge_index, params).astype(np.float32)


# revision 14
# speedup vs baseline: 1.0056x; 1.0056x over previous
"""Trainium2 Bass kernel for a 2-layer GAT (PyG GATConv semantics).

Strategy (8 NeuronCores, SPMD, 2 launches = 1 per GAT layer):
  - Destinations sharded across cores (6272 per core incl. padding dsts),
    destinations degree-sorted so per-tile slot grids pad tightly.
  - NO device-side gather. The host slot-expands the layer input:
    xTsl[:, b*128 + j] = x^T column of the source of edge slot (b, j),
    where block b = (tile t, slot level l) and partition j = destination
    lane. Slot level 0 is the self-loop (PyG add_self_loops); levels
    1..deg are the in-edges; the rest are zero-padded (masked).
  - Each 128-column block becomes one PE matmul lhsT against
    wext = [W | W@a_src^T | W@a_dst^T], producing full per-edge rows
    [h | alpha_src | alpha_dst-of-src] directly in PSUM -- the same
    trick the previous version used for self-loop rows only, now for
    every edge. PSUM blocks are copied (batched per bank) to SBUF.
  - Attention: e = alpha_src(slot) + alpha_dst(dst) (dst alphas from a
    small per-tile matmul over own columns), w = exp(lrelu(e)) * mask;
    softmax is deferred: DVE reduces w and w*h over the slot axis, then
    one reciprocal multiply normalizes; + bias (+ relu for layer 1).
  - Between layers the host assembles h1, casts to fp16 and re-expands
    the SAME slot grid (graph is static), so layer 2 is identical with
    H=1, C=64.
"""

import sys

for _p in ("/opt/trn_rl_repo", "/root/.axon_site/_ro/trn_rl_repo"):
    if _p not in sys.path:
        sys.path.insert(0, _p)

import os
from contextlib import ExitStack

import ml_dtypes
import numpy as np

import concourse.tile as tile
from concourse import bacc, mybir
from concourse.bass_utils import run_bass_kernel_spmd

# set GAT_TRACE=1 to profile each launch; exec times land in LAST_EXEC_NS
LAST_EXEC_NS = []
LAST_RES = []

CFG = {
    "group": 16,       # max tiles per group
    "blk_budget": 128,  # max T*L blocks per group (SBUF bound)
    "xsl_bufs": 2,
    "gpool_bufs": 2,
    "epool_bufs": 3,
    "psum_bufs": 3,  # bpsum tiles are 2 banks each
    "opsum_bufs": 2,
}

f32 = mybir.dt.float32
f16 = mybir.dt.float16
f8 = mybir.dt.float8e4
np_f8 = ml_dtypes.float8_e4m3

P = 128
NEG_SLOPE = 0.2
N_NODES = 50000
N_CORES = 8


# ---------------------------------------------------------------- host routing


class SlotPlan:
    """Destination-sharded slot grid; slot 0 = self-loop, then in-edges."""

    def __init__(self, src, dst, n_nodes, n_cores, group, blk_budget):
        self.n_nodes = n_nodes
        self.n_cores = n_cores
        self.dpc = int(np.ceil(n_nodes / n_cores / P)) * P
        self.nt = self.dpc // P
        nt = self.nt

        src = np.asarray(src, dtype=np.int64)
        dst = np.asarray(dst, dtype=np.int64)

        self.cores = []
        Ls = np.zeros(nt, np.int64)
        for c in range(n_cores):
            lo, hi = c * self.dpc, (c + 1) * self.dpc
            m = (dst >= lo) & (dst < hi)
            d_loc = (dst[m] - lo).astype(np.int64)
            s = src[m].astype(np.int64)
            order = np.argsort(d_loc, kind="stable")
            d_loc, s = d_loc[order], s[order]
            deg = np.bincount(d_loc, minlength=self.dpc)
            offs = np.zeros(self.dpc + 1, np.int64)
            np.cumsum(deg, out=offs[1:])
            perm = np.argsort(-deg, kind="stable").astype(np.int64)
            self.cores.append(dict(deg=deg, offs=offs, srcs=s, perm=perm))
            pt = deg[perm].reshape(nt, P)
            np.maximum(Ls, pt.max(axis=1) + 1, out=Ls)  # +1 self slot

        # SPMD-uniform groups: (g0, T) tiles sharing slot depth Lg
        self.groups = []
        t0 = 0
        while t0 < nt:
            T = 1
            while (
                T < group and t0 + T < nt
                and (T + 1) * int(Ls[t0:t0 + T + 1].max()) <= blk_budget
            ):
                T += 1
            self.groups.append((t0, T))
            t0 += T
        self.Lg = {g0: int(Ls[g0:g0 + T].max()) for g0, T in self.groups}
        self.n_blocks = sum(T * self.Lg[g0] for g0, T in self.groups)
        # edge blocks exclude the self slot (l=0): sourced from xtown
        self.n_eblocks = sum(T * (self.Lg[g0] - 1) for g0, T in self.groups)
        self.S = self.n_eblocks * P  # xsl columns per core (edges only)
        # block start offsets per group: full grid and edge-only grid
        self.gblk = {}
        self.geblk = {}
        b = eb = 0
        for g0, T in self.groups:
            self.gblk[g0] = b
            self.geblk[g0] = eb
            b += T * self.Lg[g0]
            eb += T * (self.Lg[g0] - 1)

        # per-core edge-slot->node map (-1 = pad) and validity mask
        self.col_node = []
        self.masks = []
        for c in range(n_cores):
            st = self.cores[c]
            deg, offs, srcs, perm = (st["deg"], st["offs"], st["srcs"],
                                     st["perm"])
            cn = np.full((self.n_eblocks, P), -1, np.int64)
            mask = np.zeros((self.n_blocks, P), np.float16)
            for g0, T in self.groups:
                L = self.Lg[g0]
                b0 = self.gblk[g0]
                e0 = self.geblk[g0]
                for t in range(T):
                    dsts = perm[(g0 + t) * P:(g0 + t + 1) * P]
                    mask[b0 + t * L, :] = 1.0  # self slot always valid
                    for j in range(P):
                        d = dsts[j]
                        dd = deg[d]
                        if dd:
                            o = offs[d]
                            cn[e0 + t * (L - 1):e0 + t * (L - 1) + dd, j] = \
                                srcs[o:o + dd]
                            mask[b0 + t * L + 1:b0 + t * L + 1 + dd, j] = 1.0
            self.col_node.append(cn)
            self.masks.append(np.ascontiguousarray(mask.T))  # [P, n_blocks]

    def expand(self, core, x_t):
        """[128, S] f8: x^T columns in edge-slot order; pad -> 0."""
        cn = self.col_node[core].reshape(-1)
        out = np.zeros((x_t.shape[0], cn.size), np_f8)
        ok = cn >= 0
        out[:, ok] = x_t[:, cn[ok]].astype(np_f8)
        return out

    def xtown(self, core, x_t):
        """[128, dpc] f16: own dst columns (A-order) for alpha_dst."""
        st = self.cores[core]
        node = core * self.dpc + st["perm"]
        valid = node < self.n_nodes
        out = np.zeros((x_t.shape[0], self.dpc), np.float16)
        out[:, valid] = x_t[:, node[valid]]
        return out

    def unpermute(self, core_outs, fout):
        full = np.zeros((self.n_nodes, fout), np.float32)
        for c, arr in enumerate(core_outs):
            node = c * self.dpc + self.cores[c]["perm"]
            m = node < self.n_nodes
            full[node[m]] = arr[m]
        return full


# ------------------------------------------------------------- device program


def build_layer_program(plan: SlotPlan, n_heads, ch, relu, n_cores):
    """One GAT layer over host-expanded slot columns. Returns compiled Bacc."""
    outf = n_heads * ch
    rowv = outf + n_heads  # built rows: [h | alpha_src]
    wcols = outf + 2 * n_heads  # wext input: [W | a_src | a_dst]
    nt = plan.nt
    H, C = n_heads, ch
    bank_blocks = 512 // rowv   # blocks per 512-col psum bank
    seg = bank_blocks * rowv    # cols used per bank
    per_tile = 2 * bank_blocks  # blocks per 2-bank psum tile

    nc = bacc.Bacc(
        "TRN2",
        target_bir_lowering=False,
        debug=False,
        num_devices=n_cores,
    )
    xsl = nc.dram_tensor("xsl", [P, plan.S], f8, kind="ExternalInput").ap()
    xtown = nc.dram_tensor("xtown", [P, plan.dpc], f16,
                           kind="ExternalInput").ap()
    wext = nc.dram_tensor("wext", [P, wcols], f16, kind="ExternalInput").ap()
    maskin = nc.dram_tensor("mask", [P, plan.n_blocks], f16,
                            kind="ExternalInput").ap()
    bias = nc.dram_tensor("bias", [P, outf], f32, kind="ExternalInput").ap()
    ident_in = nc.dram_tensor("ident", [P, P], f16, kind="ExternalInput").ap()
    out = nc.dram_tensor("out", [plan.dpc, outf], f32,
                         kind="ExternalOutput").ap()
    tiles_per_bank = 512 // outf  # num-psum tiles per bank

    with tile.TileContext(nc) as tc, ExitStack() as ctx:
        const = ctx.enter_context(tc.tile_pool(name="const", bufs=1))
        xpool = ctx.enter_context(
            tc.tile_pool(name="xpool", bufs=CFG["xsl_bufs"]))
        gpool = ctx.enter_context(
            tc.tile_pool(name="gpool", bufs=CFG["gpool_bufs"]))
        epool = ctx.enter_context(
            tc.tile_pool(name="epool", bufs=CFG["epool_bufs"]))
        bpsum = ctx.enter_context(
            tc.tile_pool(name="bpsum", bufs=CFG["psum_bufs"], space="PSUM"))
        npsum = ctx.enter_context(
            tc.tile_pool(name="npsum", bufs=CFG["opsum_bufs"], space="PSUM"))

        ident = const.tile([P, P], f16)
        nc.sync.dma_start(out=ident[:], in_=ident_in[:])
        wext_sb = const.tile([P, wcols], f16)
        nc.sync.dma_start(out=wext_sb[:], in_=wext[:])
        bias_sb = const.tile([P, outf], f32)
        nc.sync.dma_start(out=bias_sb[:], in_=bias[:])
        mask_sb = const.tile([P, plan.n_blocks], f16)
        nc.sync.dma_start(out=mask_sb[:], in_=maskin[:])
        xtown_sb = const.tile([P, plan.dpc], f16)
        nc.sync.dma_start(out=xtown_sb[:], in_=xtown[:])

        # ---- OWND: alpha_dst of own dsts [P, nt*H] (one psum bank)
        OWND = const.tile([P, nt * H], f16)
        ps_d = npsum.tile([P, 512], f32, space="PSUM", tag="nps")
        for k in range(nt):
            nc.tensor.matmul(
                out=ps_d[:, k * H:(k + 1) * H],
                lhsT=xtown_sb[:, k * P:(k + 1) * P],
                rhs=wext_sb[:, outf + H:outf + 2 * H],
                start=True, stop=True,
            )
        nc.vector.tensor_copy(out=OWND[:], in_=ps_d[:, :nt * H])
        OWND3 = OWND[:].rearrange("p (t h) -> p t h", t=nt, h=H)

        # ---- per-group pipeline
        for g0, T in plan.groups:
            L = plan.Lg[g0]
            nblk = T * L
            b0 = plan.gblk[g0]
            ne = T * (L - 1)
            e0 = plan.geblk[g0]

            xch = xpool.tile([P, ne * P], f8, tag="xch")
            nc.sync.dma_start(
                out=xch[:], in_=xsl[:, e0 * P:(e0 + ne) * P])

            G = gpool.tile([P, nblk * rowv], f16, tag="G")
            # build rows: one matmul per block, batched per psum bank
            b = 0
            copy_tog = 0
            while b < nblk:
                bn = min(per_tile, nblk - b)
                ps = bpsum.tile([P, 1024], f32, space="PSUM", tag="bps")
                for k in range(bn):
                    bank, off = divmod(k, bank_blocks)
                    o0 = bank * 512 + off * rowv
                    t_k, l_k = divmod(b + k, L)
                    if l_k == 0:  # self slot: f16 own column
                        lhsT = xtown_sb[:, (g0 + t_k) * P:(g0 + t_k + 1) * P]
                    else:
                        lhsT = xch[:, (t_k * (L - 1) + l_k - 1) * P:
                                   (t_k * (L - 1) + l_k) * P]
                    nc.tensor.matmul(
                        out=ps[:, o0:o0 + rowv],
                        lhsT=lhsT,
                        rhs=wext_sb[:, :rowv],
                        start=True, stop=True,
                    )
                eng = nc.vector.tensor_copy if copy_tog % 4 == 0 else None
                if bn == per_tile:
                    dst = G[:, b * rowv:(b + bn) * rowv].rearrange(
                        "p (s v) -> p s v", s=2, v=seg)
                    srcv = ps[:].rearrange(
                        "p (s q) -> p s q", s=2, q=512)[:, :, :seg]
                    if eng:
                        eng(out=dst, in_=srcv)
                    else:
                        nc.scalar.copy(dst, srcv)
                else:
                    k0 = 0
                    while k0 < bn:
                        kn = min(bank_blocks, bn - k0)
                        bank = k0 // bank_blocks
                        dst = G[:, (b + k0) * rowv:(b + k0 + kn) * rowv]
                        srcv = ps[:, bank * 512:bank * 512 + kn * rowv]
                        if eng:
                            eng(out=dst, in_=srcv)
                        else:
                            nc.scalar.copy(dst, srcv)
                        k0 += kn
                copy_tog += 1
                b += bn

            G4 = G[:].rearrange("p (t l v) -> p t l v", t=T, l=L, v=rowv)

            # ---- attention weights w = exp(lrelu(a_src + a_dst)) * mask
            E = epool.tile([P, T * L * H], f16, tag="E")
            E4 = E[:].rearrange("p (t l h) -> p t l h", t=T, l=L, h=H)
            nc.vector.tensor_tensor(
                out=E4,
                in0=G4[:, :, :, outf:outf + H],
                in1=OWND3[:, g0:g0 + T, :].unsqueeze(2)
                .to_broadcast([P, T, L, H]),
                op=mybir.AluOpType.add,
            )
            W = epool.tile([P, T * L * H], f16, tag="W")
            nc.vector.scalar_tensor_tensor(
                out=W[:], in0=E[:], scalar=NEG_SLOPE, in1=E[:],
                op0=mybir.AluOpType.mult, op1=mybir.AluOpType.max,
            )
            nc.scalar.activation(W[:], W[:], mybir.ActivationFunctionType.Exp)
            W4 = W[:].rearrange("p (t l h) -> p t l h", t=T, l=L, h=H)
            nc.vector.tensor_tensor(
                out=W4, in0=W4,
                in1=mask_sb[:, b0:b0 + nblk]
                .rearrange("p (t l) -> p t l", t=T, l=L)
                .unsqueeze(3).to_broadcast([P, T, L, H]),
                op=mybir.AluOpType.mult,
            )

            # ---- denominators + reciprocal
            den = epool.tile([P, T * H], f32, tag="den")
            den3 = den[:].rearrange("p (t h) -> p t h", t=T, h=H)
            nc.vector.tensor_reduce(
                out=den3, in_=W4.transpose([0, 1, 3, 2]),
                axis=mybir.AxisListType.X, op=mybir.AluOpType.add,
            )
            rec = epool.tile([P, T * H], f32, tag="rec")
            nc.vector.reciprocal(rec[:], den[:])
            rec3 = rec[:].rearrange("p (t h) -> p t h", t=T, h=H)

            # ---- weighted sum of h over slots: scale on DVE, reduce on PE
            gh4 = G4[:, :, :, :outf].rearrange(
                "p t l (c h) -> p t l c h", c=C, h=H)
            nc.vector.tensor_tensor(
                out=gh4, in0=gh4,
                in1=W4.unsqueeze(3).to_broadcast([P, T, L, C, H]),
                op=mybir.AluOpType.mult,
            )
            osb = epool.tile([P, T * outf], f32, tag="osb")
            osb3 = osb[:].rearrange("p (t f) -> p t f", t=T, f=outf)
            G3 = G[:].rearrange("p (b v) -> p b v", b=T * L, v=rowv)
            t0 = 0
            while t0 < T:
                tn = min(tiles_per_bank, T - t0)
                ps = npsum.tile([P, 512], f32, space="PSUM", tag="nps")
                for tt in range(tn):
                    for j in range(L):
                        nc.tensor.matmul(
                            out=ps[:, tt * outf:(tt + 1) * outf],
                            lhsT=ident[:],
                            rhs=G3[:, (t0 + tt) * L + j, :outf],
                            start=(j == 0), stop=(j == L - 1),
                        )
                # normalize from PSUM: osb = num * (1/den)
                nc.vector.tensor_tensor(
                    out=osb3[:, t0:t0 + tn, :].rearrange(
                        "p t (c h) -> p t c h", c=C, h=H),
                    in0=ps[:, :tn * outf].rearrange(
                        "p (t c h) -> p t c h", t=tn, c=C, h=H),
                    in1=rec3[:, t0:t0 + tn, :].unsqueeze(2)
                    .to_broadcast([P, tn, C, H]),
                    op=mybir.AluOpType.mult,
                )
                t0 += tn

            # ---- bias (+ relu), write out
            nc.vector.tensor_tensor(
                out=osb3, in0=osb3,
                in1=bias_sb[:].unsqueeze(1).to_broadcast([P, T, outf]),
                op=mybir.AluOpType.add,
            )
            if relu:
                nc.scalar.activation(osb[:], osb[:],
                                     mybir.ActivationFunctionType.Relu)
            nc.sync.dma_start(
                out=out[g0 * P:(g0 + T) * P, :].rearrange(
                    "(t p) f -> p t f", t=T),
                in_=osb3,
            )

    nc.compile()
    return nc


# ------------------------------------------------------------------ execution


def _prep_wext(W, att_src, att_dst):
    """[fin, outf + 2H] fp16: [W (c-major cols) | W@a_src^T | W@a_dst^T]."""
    H, C = att_src.shape
    fin = W.shape[0]
    Wr = W.reshape(fin, H, C)
    a_s = np.einsum("fhc,hc->fh", Wr, att_src)
    a_d = np.einsum("fhc,hc->fh", Wr, att_dst)
    Wi = Wr.transpose(0, 2, 1).reshape(fin, H * C)  # (c, h) column order
    return np.concatenate([Wi, a_s, a_d], axis=1).astype(np.float16)


def _interleave_cols(v, H, C):
    return np.asarray(v, np.float32).reshape(H, C).T.reshape(H * C)


def _deinterleave(arr, H, C):
    n = arr.shape[0]
    return arr.reshape(n, C, H).transpose(0, 2, 1).reshape(n, H * C)


def run_layer(plan, nc, x_t, W, att_src, att_dst, b, n_cores):
    H, C = att_src.shape
    outf = H * C
    wext = _prep_wext(np.asarray(W, np.float32),
                      np.asarray(att_src, np.float32),
                      np.asarray(att_dst, np.float32))
    bias = np.broadcast_to(_interleave_cols(b, H, C), (P, outf)).copy()
    ident = np.eye(P, dtype=np.float16)
    in_maps = [
        {"xsl": plan.expand(c, x_t), "xtown": plan.xtown(c, x_t),
         "wext": wext, "bias": bias, "mask": plan.masks[c], "ident": ident}
        for c in range(n_cores)
    ]
    trace = os.environ.get("GAT_TRACE", "") == "1"
    res = run_bass_kernel_spmd(nc, in_maps, list(range(n_cores)), trace=trace)
    if trace:
        LAST_EXEC_NS.append(res.exec_time_ns)
        LAST_RES.append(res)
    outs = [res.results[c]["out"] for c in range(n_cores)]
    return _deinterleave(plan.unpermute(outs, outf), H, C)


def gat_forward(x, edge_index, params, n_cores=N_CORES):
    x = np.asarray(x, np.float32)
    n = x.shape[0]
    ei = np.asarray(edge_index)

    plan = SlotPlan(ei[0], ei[1], n, n_cores, CFG["group"],
                    CFG["blk_budget"])
    W1, as1, ad1, b1, W2, as2, ad2, b2 = params

    x_t = x.T.astype(np.float16)
    nc1 = build_layer_program(plan, as1.shape[0], as1.shape[1],
                              relu=True, n_cores=n_cores)
    h = run_layer(plan, nc1, x_t, W1, as1, ad1, b1, n_cores)

    h_t = h.T.astype(np.float16)
    nc2 = build_layer_program(plan, as2.shape[0], as2.shape[1],
                              relu=False, n_cores=n_cores)
    out = run_layer(plan, nc2, h_t, W2, as2, ad2, b2, n_cores)
    return out


def kernel(x, edge_index, W1, att_src1, att_dst1, b1, W2, att_src2,
           att_dst2, b2):
    params = tuple(
        np.asarray(a, np.float32)
        for a in (W1, att_src1, att_dst1, b1, W2, att_src2, att_dst2, b2)
    )
    return gat_forward(x, edge_index, params).astype(np.float32)
